# revision 42
# baseline (speedup 1.0000x reference)
"""AdaptFormer fused kernel for one TRN2 chip (8 NeuronCores).

Strategy: pure data-parallel over batch (16 batches -> 2 per core). Each core
runs the full fusion + both transformer streams for its 2 batches. All
activations live feature-major ([D on partitions, tokens on free dim]) so every
GEMM runs weights-stationary with zero transposes; LayerNorm/softmax partition
reductions are done with ones-vector matmuls on the TensorEngine, and
[1,N]->[P,N] broadcasts with ones-stationary matmuls. Matmul dtype is float32r
(TF32): full throughput at free-dim >= 256 with ~1e-4 relative rounding.
"""

import sys

sys.path.insert(0, "/opt/trn_rl_repo")

import math
import numpy as np

import concourse.bass as bass
import concourse.bacc as bacc
import concourse.mybir as mybir
import concourse.tile as tile
from concourse.bass_utils import run_bass_kernel_spmd
from concourse.masks import make_identity

F32 = mybir.dt.float32
F32R = mybir.dt.float32r
BF16 = mybir.dt.bfloat16
AF = mybir.ActivationFunctionType
OP = mybir.AluOpType

D = 768
KC = 6          # D / 128
H = 12
DH = 64
L = 32
DFF = 3072
ADIM = 64
EPS = 1e-6
NB = 2          # batches per core
NX = 512        # x tokens per batch
NYR = 196       # real y tokens per batch
NYP = 256       # padded y tokens per batch
RS2 = 1.0 / math.sqrt(D)   # fusion attention scale
RS8 = 1.0 / math.sqrt(DH)  # mhsa scale


def _dma_r(nc, dst_ap, src_ap):
    nc.sync.dma_start(out=dst_ap, in_=src_ap.bitcast(F32R))


class Ctx:
    pass


def _declare_params(nc):
    P = Ctx()
    dp = nc.declare_dram_parameter
    P.xfm = dp("xfm", [NB, D, NX], F32, isOutput=False)
    P.yfm = dp("yfm", [NB, D, NYP], F32, isOutput=False)
    P.xrm = dp("xrm", [NB, NX, D], F32, isOutput=False)
    P.yrm = dp("yrm", [NB, NYR, D], F32, isOutput=False)
    P.latfm = dp("latfm", [D, L], F32, isOutput=False)
    P.ident = dp("ident", [128, 128], F32, isOutput=False)
    P.ones = dp("ones", [128, 128], F32, isOutput=False)
    P.sa = dp("sa", [1, 1], F32, isOutput=False)
    P.sv = dp("sv", [1, 1], F32, isOutput=False)
    for p in ("s", "r"):
        for nm, shp in (
            ("qkvb", [128, 18]), ("projb", [128, 6]),
            ("fc1b", [128, 24]), ("fc2b", [128, 6]),
            ("downb", [ADIM, 1]), ("downb2", [ADIM, 1]),
            ("ln1g", [128, 6]), ("ln1b", [128, 6]),
            ("ln2g", [128, 6]), ("ln2b", [128, 6]),
        ):
            setattr(P, f"{p}_{nm}", dp(f"{p}_{nm}", shp, F32, isOutput=False))
        for nm, shp in (
            ("qkvwT", [D, 3 * D]), ("projwT", [D, D]),
            ("fc1wT", [D, DFF]), ("fc2wT", [DFF, D]),
            ("downwT", [D, ADIM]), ("upwT", [ADIM, D]),
        ):
            setattr(P, f"{p}_{nm}", dp(f"{p}_{nm}", shp, BF16, isOutput=False))
    P.xout = dp("xout", [NB, D, NX], F32, isOutput=True)
    P.yout = dp("yout", [NB, D, NYR], F32, isOutput=True)
    return P


def build(stage="full"):
    nc = bacc.Bacc(None, target_bir_lowering=False)
    P = _declare_params(nc)
    alp = nc.allow_low_precision("f32r rounding is intended")
    alp.__enter__()

    with tile.TileContext(nc) as tc:
        _build_body(nc, tc, P, stage)
    alp.__exit__(None, None, None)
    nc.finalize()
    return nc


def _build_body(nc, tc, P, stage):
    from contextlib import ExitStack

    ctx = ExitStack()
    const = ctx.enter_context(tc.tile_pool(name="const", bufs=1))
    act = ctx.enter_context(tc.tile_pool(name="act", bufs=1))
    tmp = ctx.enter_context(tc.tile_pool(name="tmp", bufs=2))
    pg = ctx.enter_context(tc.tile_pool(name="pg", bufs=4, space="PSUM"))
    po = ctx.enter_context(tc.tile_pool(name="po", bufs=2, space="PSUM"))
    pst = ctx.enter_context(tc.tile_pool(name="pst", bufs=2, space="PSUM"))

    # ---- constants ----
    ident = const.tile([128, 128], F32, name="ident")
    nc.sync.dma_start(out=ident[:], in_=P.ident[:, :])
    ones = const.tile([128, 128], F32R, name="ones")
    _dma_r(nc, ones[:], P.ones[:, :])
    sa_t = const.tile([1, 1], F32, name="sa_t")
    nc.sync.dma_start(out=sa_t[:], in_=P.sa[:, :])
    sv_t = const.tile([1, 1], F32, name="sv_t")
    nc.sync.dma_start(out=sv_t[:], in_=P.sv[:, :])
    eps_t = const.tile([1, 1], F32, name="eps_t")
    nc.vector.memset(eps_t[:], EPS)

    bias = {}
    for p in ("s", "r"):
        for nm in ("qkvb", "projb", "fc1b", "fc2b", "ln1g", "ln1b",
                   "ln2g", "ln2b"):
            h = getattr(P, f"{p}_{nm}")
            t = const.tile(list(h.shape), F32, name=f"{p}_{nm}_t")
            nc.sync.dma_start(out=t[:], in_=h[:, :])
            bias[f"{p}_{nm}"] = t
        for nm in ("downb", "downb2"):
            h = getattr(P, f"{p}_{nm}")
            t = const.tile([ADIM, 1], F32, name=f"{p}_{nm}_t")
            nc.sync.dma_start(out=t[:], in_=h[:, :])
            bias[f"{p}_{nm}"] = t

    lat = [const.tile([128, L], F32R, name=f"lat{k}") for k in range(KC)]
    for k in range(KC):
        _dma_r(nc, lat[k][:], P.latfm[k * 128:(k + 1) * 128, :])

    # ---- stream state tiles (xfm / yfm hold the running residual) ----
    xfm = [[act.tile([128, NX], F32R, name=f"xfm{b}_{k}")
            for k in range(KC)] for b in range(NB)]
    yfm = [act.tile([128, 2 * NYP], F32R, name=f"yfm_{k}")
           for k in range(KC)]
    for b in range(NB):
        for k in range(KC):
            _dma_r(nc, xfm[b][k][:], P.xfm[b, k * 128:(k + 1) * 128, :])
            _dma_r(nc, yfm[k][:, b * NYP:(b + 1) * NYP],
                   P.yfm[b, k * 128:(k + 1) * 128, :])

    # V' ring: [128, 65] tiles, col 64 = ones (written once)
    vring = [const.tile([128, DH + 1], F32R, name=f"vr{i}") for i in range(5)]
    for i in range(5):
        _dma_r(nc, vring[i][:, DH:DH + 1], P.ones[:, 0:1])
    vslot = [0]

    # ================= FUSION =================
    with ExitStack() as fctx:
        fus = fctx.enter_context(tc.tile_pool(name="fus", bufs=1))
        rmp = fctx.enter_context(tc.tile_pool(name="rmp", bufs=1))
        ftmp = fctx.enter_context(tc.tile_pool(name="ftmp", bufs=2))

        # row-major copies of x/y for the P1^T @ concat_rm matmul
        xrm = [[rmp.tile([128, D], F32R, name=f"xrm{b}_{r}")
                for r in range(4)] for b in range(NB)]
        yrm = [[rmp.tile([128, D], F32R, name=f"yrm{b}_{r}")
                for r in range(2)] for b in range(NB)]
        for b in range(NB):
            for r in range(4):
                _dma_r(nc, xrm[b][r][:], P.xrm[b, r * 128:(r + 1) * 128, :])
            _dma_r(nc, yrm[b][0][:], P.yrm[b, 0:128, :])
            _dma_r(nc, yrm[b][1][0:68, :], P.yrm[b, 128:196, :])

        # --- S1 = lat @ concat^T, exp, per-batch softmax sums ---
        es1 = fus.tile([L, 1536], F32R, name="es1")
        sums = [[fus.tile([L, 1], F32, name=f"s1s{b}_{i}") for i in range(2)]
                for b in range(NB)]
        for b in range(NB):
            s1x = pg.tile([L, NX], F32, name="s1x", tag="pg")
            for k in range(KC):
                nc.tensor.matmul(s1x[:], lat[k][:], xfm[b][k][:],
                                 start=(k == 0), stop=(k == KC - 1))
            nc.scalar.activation(es1[:, b * NX:(b + 1) * NX], s1x[:], AF.Exp,
                                 scale=RS2, accum_out=sums[b][0][:])
            s1y = pg.tile([L, NYP], F32, name="s1y", tag="pg")
            for k in range(KC):
                nc.tensor.matmul(s1y[:], lat[k][:],
                                 yfm[k][:, b * NYP:(b + 1) * NYP],
                                 start=(k == 0), stop=(k == KC - 1))
            yo = 1024 + b * NYP
            nc.scalar.activation(es1[:, yo:yo + NYR], s1y[:, 0:NYR], AF.Exp,
                                 scale=RS2, accum_out=sums[b][1][:])

        # P1 = exp/sum (per batch), in place
        for b in range(NB):
            st = fus.tile([L, 1], F32, name=f"s1t{b}")
            nc.vector.tensor_tensor(st[:], sums[b][0][:], sums[b][1][:], OP.add)
            rec = fus.tile([L, 1], F32, name=f"s1r{b}")
            nc.vector.reciprocal(rec[:], st[:])
            nc.vector.tensor_scalar(
                es1[:, b * NX:(b + 1) * NX], es1[:, b * NX:(b + 1) * NX],
                rec[:], None, OP.mult)
            yo = 1024 + b * NYP
            nc.vector.tensor_scalar(
                es1[:, yo:yo + NYR], es1[:, yo:yo + NYR],
                rec[:], None, OP.mult)

        # --- fused = P1 @ concat  (row-major [L, 768] per batch) ---
        fusrm = [fus.tile([L, D], F32R, name=f"fusrm{b}") for b in range(NB)]
        for b in range(NB):
            # P1^T chunks: x: 4x[rows=128], y: [128],[68]
            p1t = []
            chunks = [(b * NX + c * 128, 128, xrm[b][c]) for c in range(4)]
            yo = 1024 + b * NYP
            chunks.append((yo, 128, yrm[b][0]))
            chunks.append((yo + 128, 68, yrm[b][1]))
            for ci, (off, cnt, _) in enumerate(chunks):
                tp = pg.tile([128, L], F32, name="p1tp", tag="pg")
                nc.tensor.matmul(tp[0:cnt, :],
                                 es1[:, off:off + cnt].bitcast(F32),
                                 ident[0:L, 0:L], is_transpose=True)
                t = ftmp.tile([128, L], F32R, name="p1t", tag="p1t", bufs=6)
                nc.scalar.activation(t[0:cnt, :], tp[0:cnt, :], AF.Identity)
                p1t.append(t)
            for dc, dw in ((0, 512), (512, 256)):
                fps = pg.tile([L, dw], F32, name="fps", tag="pg")
                for ci, (off, cnt, rm) in enumerate(chunks):
                    nc.tensor.matmul(fps[:], p1t[ci][0:cnt, :],
                                     rm[0:cnt, dc:dc + dw],
                                     start=(ci == 0), stop=(ci == len(chunks) - 1))
                nc.scalar.activation(fusrm[b][:, dc:dc + dw], fps[:],
                                     AF.Identity)

        # fused feature-major [D, L]
        fusfm = [[fus.tile([128, L], F32R, name=f"fusfm{b}_{k}")
                  for k in range(KC)] for b in range(NB)]
        for b in range(NB):
            for k in range(KC):
                tp = pg.tile([128, L], F32, name="ffmp", tag="pg")
                nc.tensor.matmul(tp[:], fusrm[b][:, k * 128:(k + 1) * 128].bitcast(F32),
                                 ident[0:L, 0:L], is_transpose=True)
                nc.scalar.activation(fusfm[b][k][:], tp[:], AF.Identity)

        # --- x += sa * shattn(x, fused, fused);  y += sv * shattn(y, ...) ---
        def latent_attn(fm_aps, ncols, scale_t, b):
            sps = pg.tile([L, ncols], F32, name="sps", tag="pg")
            for k in range(KC):
                nc.tensor.matmul(sps[:], fusfm[b][k][:], fm_aps[k],
                                 start=(k == 0), stop=(k == KC - 1))
            es = ftmp.tile([L, ncols], F32R, name="es23", tag="es23")
            nc.scalar.activation(es[:], sps[:], AF.Exp, scale=RS2)
            ssum = pst.tile([1, ncols], F32, name="ssum", tag="pst")
            nc.tensor.matmul(ssum[:], ones[0:L, 0:1], es[:],
                             start=True, stop=True)
            recf = ftmp.tile([1, ncols], F32, name="recf", tag="recf")
            nc.vector.reciprocal_approx_fast(recf[:], ssum[:])
            rec = ftmp.tile([1, ncols], F32R, name="rec23", tag="rec23")
            nc.vector.tensor_scalar(rec[:], recf[:], scale_t[:], None, OP.mult)
            rbp = pg.tile([128, ncols], F32, name="rbp", tag="pg")
            nc.tensor.matmul(rbp[:], ones[0:1, :], rec[:], start=True, stop=True)
            rb = ftmp.tile([128, ncols], F32, name="rb23", tag="rb23")
            nc.scalar.activation(rb[:], rbp[:], AF.Identity)
            for k in range(KC):
                ops = pg.tile([128, ncols], F32, name="ops", tag="pg")
                nc.tensor.matmul(ops[:], fusrm[b][:, k * 128:(k + 1) * 128],
                                 es[:], start=True, stop=True)
                tv = ftmp.tile([128, ncols], F32, name="tv23", tag="tv23")
                nc.vector.tensor_tensor(tv[:], ops[:], rb[:], OP.mult)
                nc.vector.tensor_tensor(fm_aps[k], fm_aps[k].bitcast(F32),
                                        tv[:], OP.add)

        for b in range(NB):
            latent_attn([xfm[b][k][:] for k in range(KC)], NX, sa_t, b)
            latent_attn([yfm[k][:, b * NYP:(b + 1) * NYP] for k in range(KC)],
                        NYP, sv_t, b)

    if stage == "fusion":
        for b in range(NB):
            for k in range(KC):
                nc.sync.dma_start(out=P.xout[b, k * 128:(k + 1) * 128, :],
                                  in_=xfm[b][k][:].bitcast(F32))
                nc.sync.dma_start(
                    out=P.yout[b, k * 128:(k + 1) * 128, :],
                    in_=yfm[k][:, b * NYP:b * NYP + NYR].bitcast(F32))
        ctx.close()
        return

    # ================= STREAMS =================
    wpool = ctx.enter_context(tc.tile_pool(name="wp", bufs=9))
    hpool = ctx.enter_context(tc.tile_pool(name="hp", bufs=12))
    qkvp = ctx.enter_context(tc.tile_pool(name="qkvp", bufs=18))
    mlpp = ctx.enter_context(tc.tile_pool(name="mlpp", bufs=50))
    ofmp = ctx.enter_context(tc.tile_pool(name="ofmp", bufs=13))

    def layernorm(chunks, g_t, b_t, out_dt=BF16):
        """chunks: list of (tiles, ncols). returns list of h tile lists."""
        out = []
        for tiles, ncols in chunks:
            mps = pst.tile([1, ncols], F32, name="mps", tag="pst")
            for k in range(KC):
                nc.tensor.matmul(mps[:], ones[:, 0:1], tiles[k][:],
                                 start=(k == 0), stop=(k == KC - 1))
            s2ps = pst.tile([1, ncols], F32, name="s2ps", tag="pst")
            for k in range(KC):
                sq = tmp.tile([128, ncols], F32R, name="sq", tag="sq", bufs=1)
                nc.vector.tensor_tensor(sq[:], tiles[k][:].bitcast(F32),
                                        tiles[k][:].bitcast(F32), OP.mult)
                nc.tensor.matmul(s2ps[:], ones[:, 0:1], sq[:],
                                 start=(k == 0), stop=(k == KC - 1))
            mb = tmp.tile([1, ncols], F32R, name="mb", tag="lnt1")
            t2 = tmp.tile([1, ncols], F32, name="t2", tag="lnt2")
            sdt = tmp.tile([1, ncols], F32R, name="sdt", tag="lnsd")
            nc.vector.tensor_scalar(mb[:], mps[:], 1.0 / D, None, OP.mult)
            nc.vector.tensor_tensor(t2[:], mb[:].bitcast(F32),
                                    mb[:].bitcast(F32), OP.mult)
            nc.vector.scalar_tensor_tensor(t2[:], s2ps[:], 1.0 / D, t2[:],
                                           OP.mult, OP.subtract)
            nc.scalar.activation(sdt[:], t2[:], AF.Sqrt, bias=eps_t[:])
            Mps = pg.tile([128, ncols], F32, name="Mps", tag="pg")
            nc.tensor.matmul(Mps[:], ones[0:1, :], mb[:], start=True, stop=True)
            Sps = pg.tile([128, ncols], F32, name="Sps", tag="pg")
            nc.tensor.matmul(Sps[:], ones[0:1, :], sdt[:], start=True, stop=True)
            Ab = tmp.tile([128, ncols], F32, name="Ab", tag="lnAb", bufs=2)
            nc.vector.reciprocal_approx_fast(Ab[:], Sps[:])
            hts = []
            for k in range(KC):
                z = tmp.tile([128, ncols], F32, name="z", tag="z", bufs=1)
                nc.vector.tensor_tensor(z[:], tiles[k][:].bitcast(F32), Mps[:],
                                        OP.subtract)
                nc.vector.tensor_tensor(z[:], z[:], Ab[:], OP.mult)
                ht = hpool.tile([128, ncols], out_dt, name="ht", tag="ht")
                nc.vector.tensor_scalar(ht[:], z[:], g_t[:, k:k + 1],
                                        b_t[:, k:k + 1], OP.mult, OP.add)
                hts.append(ht)
            out.append(hts)
        return out

    def gemm(wT, F, src_chunks, evict, wtag="ws"):
        """out[f, n] = sum_k wT[k, f] * src[k, n]; evict(ps, fc, ci).
        Weights stream in [128, 512]-wide tiles (4 f-chunks per DMA).
        src_chunks entries: (tiles, dn, vw) with vw mapping a [128, nc]
        tile AP to the dense moving-operand view of free size dn."""
        for fb in range(0, F, 512):
            fbw = min(512, F - fb)
            wts = []
            for k in range(KC):
                wt = wpool.tile([128, 512], BF16, name=f"w{fb}_{k}", tag=wtag)
                nc.sync.dma_start(out=wt[:, 0:fbw],
                                  in_=wT[k * 128:(k + 1) * 128, fb:fb + fbw])
                wts.append(wt)
            for fs in range(0, fbw, 128):
                fc = (fb + fs) // 128
                for ci, (tiles, dn, vw) in enumerate(src_chunks):
                    ps = pg.tile([128, dn], F32, name="gps", tag="pg")
                    for k in range(KC):
                        nc.tensor.matmul(ps[:], wts[k][:, fs:fs + 128],
                                         vw(tiles[k][:]),
                                         start=(k == 0), stop=(k == KC - 1))
                    evict(ps, fc, ci)

    def attention(qkv, ofm, batches):
        for h in range(H):
            po_, pslc = h // 2, slice((h % 2) * 64, (h % 2) * 64 + 64)
            pb = (h % 2) * 64
            for (boff, ncb, nreal) in batches:
                nkr = [(boff + c * 128, min(128, nreal - c * 128))
                       for c in range((nreal + 127) // 128)]
                q_ap = qkv[po_][pslc, boff:boff + ncb]
                vps = []
                for (off, cnt) in nkr:
                    vp = vring[vslot[0] % 5]
                    vslot[0] += 1
                    tp = pg.tile([128, DH], F32, name="vtp", tag="pg")
                    nc.tensor.matmul(
                        tp[0:cnt, :],
                        qkv[12 + po_][pslc, off:off + cnt].bitcast(F32),
                        ident[pb:pb + DH, pb:pb + DH], is_transpose=True)
                    nc.vector.tensor_copy(vp[0:cnt, 0:DH], tp[0:cnt, :])
                    vps.append(vp)
                ops = po.tile([DH + 1, ncb], F32, name="attops", tag="po")
                for ci, (off, cnt) in enumerate(nkr):
                    sps = pg.tile([128, ncb], F32, name="attsps", tag="pg")
                    nc.tensor.matmul(sps[0:cnt, :],
                                     qkv[6 + po_][pslc, off:off + cnt],
                                     q_ap, start=True, stop=True)
                    es = tmp.tile([128, ncb], F32R, name="attes", tag="attes", bufs=4)
                    nc.scalar.activation(es[0:cnt, :], sps[0:cnt, :], AF.Exp,
                                         scale=RS8)
                    nc.tensor.matmul(ops[:], vps[ci][0:cnt, :], es[0:cnt, :],
                                     start=(ci == 0), stop=(ci == len(nkr) - 1))
                ssb = tmp.tile([1, ncb], F32R, name="attssb", tag="attrec",
                               bufs=1)
                nc.scalar.activation(ssb[:], ops[DH:DH + 1, :], AF.Identity)
                rbp = pg.tile([DH, ncb], F32, name="attrbp", tag="pg")
                nc.tensor.matmul(rbp[:], ones[0:1, 0:DH], ssb[:],
                                 start=True, stop=True)
                rb = tmp.tile([DH, ncb], F32, name="attrb", tag="attrb",
                              bufs=1)
                nc.vector.reciprocal_approx_fast(rb[:], rbp[:])
                nc.vector.tensor_tensor(ofm[po_][pslc, boff:boff + ncb],
                                        ops[0:DH, :], rb[:], OP.mult)

    def stream(pfx, chunks, upto="full"):
        lnchunks = [(ch["st"], ch["nc"]) for ch in chunks]
        h1 = layernorm(lnchunks, bias[f"{pfx}_ln1g"], bias[f"{pfx}_ln1b"])
        wT = getattr(P, f"{pfx}_qkvwT")
        qb = bias[f"{pfx}_qkvb"]
        ofms = []
        for ci_, ch in enumerate(chunks):
            ncols = ch["nc"]
            dn = ch.get("dn", ncols)
            vw = ch.get("vw") or (lambda a: a)
            pv = ch.get("pv") or (lambda a: a)
            qkv = [qkvp.tile([128, ncols], F32R, name=f"qkv{ci_}_{j}",
                             tag="qkv") for j in range(18)]

            def ev_qkv(ps, fc, ci, qkv=qkv, qb=qb, vw=vw, pv=pv):
                nc.vector.tensor_scalar(vw(qkv[fc][:]), pv(ps[:]),
                                        qb[:, fc:fc + 1], None, OP.add)

            gemm(wT, 3 * D, [(h1[ci_], dn, vw)], ev_qkv)
            ofm = [ofmp.tile([128, ncols], BF16, name=f"ofm{ci_}_{k}",
                             tag="ofm") for k in range(KC)]
            attention(qkv, ofm, ch["bt"])
            ofms.append(ofm)

        def ev_proj(ps, fc, ci):
            ch = chunks[ci]
            vw = ch.get("vw") or (lambda a: a)
            pv = ch.get("pv") or (lambda a: a)
            nc.vector.scalar_tensor_tensor(
                vw(ch["st"][fc][:]), pv(ps[:]),
                bias[f"{pfx}_projb"][:, fc:fc + 1],
                vw(ch["st"][fc][:].bitcast(F32)), OP.add, OP.add)

        gemm(getattr(P, f"{pfx}_projwT"), D,
             [(ofms[i], chunks[i].get("dn", chunks[i]["nc"]),
               chunks[i].get("vw") or (lambda a: a))
              for i in range(len(chunks))], ev_proj)
        if upto == "attn":
            return
        h2 = layernorm(lnchunks, bias[f"{pfx}_ln2g"], bias[f"{pfx}_ln2b"])
        upT = getattr(P, f"{pfx}_upwT")
        nch = len(chunks)
        dns = [ch.get("dn", ch["nc"]) for ch in chunks]
        vws = [ch.get("vw") or (lambda a: a) for ch in chunks]
        pvs = [ch.get("pv") or (lambda a: a) for ch in chunks]
        ahs, mlps = [], []
        dwT = getattr(P, f"{pfx}_downwT")
        dwts = []
        for k in range(KC):
            dwt = wpool.tile([128, ADIM], BF16, name=f"dw{k}", tag="ws")
            nc.sync.dma_start(out=dwt[:], in_=dwT[k * 128:(k + 1) * 128, :])
            dwts.append(dwt)
        for ci_, ch in enumerate(chunks):
            dn, vw = dns[ci_], vws[ci_]
            aps = pg.tile([ADIM, dn], F32, name="aps", tag="pg")
            for k in range(KC):
                nc.tensor.matmul(aps[:], dwts[k][:], vw(h2[ci_][k][:]),
                                 start=(k == 0), stop=(k == KC - 1))
            eh = tmp.tile([ADIM, dn], F32, name="eh", tag="eh", bufs=1)
            nc.scalar.activation(eh[:], aps[:], AF.Exp, scale=-1.702,
                                 bias=bias[f"{pfx}_downb"][:])
            nc.vector.tensor_scalar(eh[:], eh[:], 1.0, None, OP.add)
            sig = tmp.tile([ADIM, dn], F32, name="sig", tag="sig", bufs=1)
            nc.vector.reciprocal_approx_fast(sig[:], eh[:])
            ah = tmp.tile([ADIM, dn], BF16, name="ah", tag="ah", bufs=2)
            nc.vector.scalar_tensor_tensor(ah[:], aps[:],
                                           bias[f"{pfx}_downb2"][:],
                                           sig[:], OP.add, OP.mult)
            ahs.append(ah)
            mlps.append([mlpp.tile([128, ch["nc"]], BF16,
                                   name=f"mlp{ci_}_{j}", tag="mlp")
                         for j in range(24)])

        def ev_fc1(ps, fc, ci):
            nc.scalar.activation(vws[ci](mlps[ci][fc][:]), pvs[ci](ps[:]),
                                 AF.Gelu,
                                 bias=bias[f"{pfx}_fc1b"][:, fc:fc + 1])

        gemm(getattr(P, f"{pfx}_fc1wT"), DFF,
             [(h2[i], dns[i], vws[i]) for i in range(nch)], ev_fc1)
        f2T = getattr(P, f"{pfx}_fc2wT")
        for fc in range(KC):
            pss = [pg.tile([128, dns[i]], F32, name=f"f2ps{i}", tag="pg")
                   for i in range(nch)]
            for kb in range(6):
                wt = wpool.tile([128, 4, 128], BF16, name=f"f2w{kb}",
                                tag="ws")
                src = f2T[kb * 512:(kb + 1) * 512,
                          fc * 128:(fc + 1) * 128]
                nc.sync.dma_start(
                    out=wt[:], in_=src.rearrange("(n p) m -> p n m", p=128))
                for j in range(4):
                    k = kb * 4 + j
                    for i in range(nch):
                        nc.tensor.matmul(pss[i][:], wt[:, j, :],
                                         vws[i](mlps[i][k][:]),
                                         start=(k == 0), stop=False)
            uwt = wpool.tile([ADIM, 128], BF16, name=f"uw{fc}", tag="ws")
            nc.sync.dma_start(out=uwt[:],
                              in_=upT[:, fc * 128:(fc + 1) * 128])
            for i in range(nch):
                nc.tensor.matmul(pss[i][:], uwt[:], ahs[i][:],
                                 start=False, stop=True)
                nc.vector.scalar_tensor_tensor(
                    vws[i](chunks[i]["st"][fc][:]), pvs[i](pss[i][:]),
                    bias[f"{pfx}_fc2b"][:, fc:fc + 1],
                    vws[i](chunks[i]["st"][fc][:].bitcast(F32)),
                    OP.add, OP.add)

    upto = "attn" if stage == "attn" else "full"
    xchunks = [{"st": xfm[b], "nc": NX, "bt": [(0, NX, NX)]}
               for b in range(NB)]
    def _yvw(a):
        return a.rearrange("p (b c) -> p b c", b=2)[:, :, 0:NYR]

    def _ypv(a):
        return a.rearrange("p (b c) -> p b c", b=2)

    ychunks = [{"st": yfm, "nc": 2 * NYP, "dn": 2 * NYR,
                "vw": _yvw, "pv": _ypv,
                "bt": [(0, NYP, NYR), (NYP, NYP, NYR)]}]
    stream("s", xchunks, upto)
    stream("r", ychunks, upto)

    # ---- outputs ----
    for b in range(NB):
        for k in range(KC):
            nc.sync.dma_start(out=P.xout[b, k * 128:(k + 1) * 128, :],
                              in_=xfm[b][k][:].bitcast(F32))
            nc.sync.dma_start(
                out=P.yout[b, k * 128:(k + 1) * 128, :],
                in_=yfm[k][:, b * NYP:b * NYP + NYR].bitcast(F32))
    ctx.close()


# ============================ host side ============================

_CACHE = {}


def _get_nc(stage="full"):
    if stage not in _CACHE:
        _CACHE[stage] = build(stage)
    return _CACHE[stage]


def _prep_core_inputs(c, x, y, latents, scale_a, scale_v, W):
    b0 = 2 * c
    xb = x[b0:b0 + 2]
    yb = y[b0:b0 + 2]
    yfm = np.zeros((NB, D, NYP), np.float32)
    yfm[:, :, :NYR] = yb.transpose(0, 2, 1)
    m = {
        "xfm": np.ascontiguousarray(xb.transpose(0, 2, 1)),
        "yfm": yfm,
        "xrm": np.ascontiguousarray(xb),
        "yrm": np.ascontiguousarray(yb),
    }
    m.update(W)
    return m


def _shared_inputs(latents, scale_a, scale_v, kw):
    W = {
        "latfm": np.ascontiguousarray(latents[0].T),
        "ident": np.eye(128, dtype=np.float32),
        "ones": np.ones((128, 128), np.float32),
        "sa": np.array([[float(scale_a[0])]], np.float32),
        "sv": np.array([[float(scale_v[0])]], np.float32),
    }
    for p, q in (("s", "spec"), ("r", "rgb")):
        g = lambda nm: np.asarray(kw[f"{q}_{nm}"], np.float32)
        sc = float(g("scale")[0])
        import ml_dtypes
        bf = ml_dtypes.bfloat16
        W[f"{p}_qkvwT"] = np.ascontiguousarray(g("qkv_w").T.astype(bf))
        W[f"{p}_projwT"] = np.ascontiguousarray(g("proj_w").T.astype(bf))
        W[f"{p}_fc1wT"] = np.ascontiguousarray(g("fc1_w").T.astype(bf))
        W[f"{p}_fc2wT"] = np.ascontiguousarray(g("fc2_w").T.astype(bf))
        W[f"{p}_downwT"] = np.ascontiguousarray(g("down_w").T.astype(bf))
        W[f"{p}_upwT"] = np.ascontiguousarray((g("up_w").T * sc).astype(bf))
        W[f"{p}_qkvb"] = np.ascontiguousarray(g("qkv_b").reshape(18, 128).T)
        W[f"{p}_projb"] = np.ascontiguousarray(g("proj_b").reshape(6, 128).T)
        W[f"{p}_fc1b"] = np.ascontiguousarray(g("fc1_b").reshape(24, 128).T)
        W[f"{p}_fc2b"] = np.ascontiguousarray(
            (g("fc2_b") + g("up_b") * sc).reshape(6, 128).T)
        W[f"{p}_downb"] = np.ascontiguousarray(
            (g("down_b") * -1.702).reshape(ADIM, 1))
        W[f"{p}_downb2"] = np.ascontiguousarray(g("down_b").reshape(ADIM, 1))
        W[f"{p}_ln1g"] = np.ascontiguousarray(g("ln1_g").reshape(6, 128).T)
        W[f"{p}_ln1b"] = np.ascontiguousarray(g("ln1_b").reshape(6, 128).T)
        W[f"{p}_ln2g"] = np.ascontiguousarray(g("ln2_g").reshape(6, 128).T)
        W[f"{p}_ln2b"] = np.ascontiguousarray(g("ln2_b").reshape(6, 128).T)
    return W


def run(stage="full", trace=False, **kw):
    x = np.asarray(kw["x"], np.float32)
    y = np.asarray(kw["y"], np.float32)
    W = _shared_inputs(np.asarray(kw["latents"], np.float32),
                       np.asarray(kw["scale_a"], np.float32),
                       np.asarray(kw["scale_v"], np.float32), kw)
    in_maps = [_prep_core_inputs(c, x, y, None, None, None, W)
               for c in range(8)]
    nc = _get_nc(stage)
    res = run_bass_kernel_spmd(nc, in_maps, core_ids=list(range(8)),
                               trace=trace)
    xs, ys = [], []
    for c in range(8):
        r = res.results[c]
        xs.append(r["xout"].transpose(0, 2, 1))
        ys.append(r["yout"].transpose(0, 2, 1))
    xo = np.concatenate(xs, axis=0)
    yo = np.concatenate(ys, axis=0)
    return (xo, yo), res


def kernel(**inputs):
    out, _ = run(stage="full", trace=False, **inputs)
    return out


# revision 44
# speedup vs baseline: 1.0309x; 1.0309x over previous
"""AdaptFormer fused kernel for one TRN2 chip (8 NeuronCores).

Strategy: pure data-parallel over batch (16 batches -> 2 per core). Each core
runs the full fusion + both transformer streams for its 2 batches. All
activations live feature-major ([D on partitions, tokens on free dim]) so every
GEMM runs weights-stationary with zero transposes; LayerNorm/softmax partition
reductions are done with ones-vector matmuls on the TensorEngine, and
[1,N]->[P,N] broadcasts with ones-stationary matmuls. Matmul dtype is float32r
(TF32): full throughput at free-dim >= 256 with ~1e-4 relative rounding.
"""

import sys

sys.path.insert(0, "/opt/trn_rl_repo")

import math
import numpy as np

import concourse.bass as bass
import concourse.bacc as bacc
import concourse.mybir as mybir
import concourse.tile as tile
from concourse.bass_utils import run_bass_kernel_spmd
from concourse.masks import make_identity

F32 = mybir.dt.float32
F32R = mybir.dt.float32r
BF16 = mybir.dt.bfloat16
AF = mybir.ActivationFunctionType
OP = mybir.AluOpType

D = 768
KC = 6          # D / 128
H = 12
DH = 64
L = 32
DFF = 3072
ADIM = 64
EPS = 1e-6
NB = 2          # batches per core
NX = 512        # x tokens per batch
NYR = 196       # real y tokens per batch
NYP = 256       # padded y tokens per batch
RS2 = 1.0 / math.sqrt(D)   # fusion attention scale
RS8 = 1.0 / math.sqrt(DH)  # mhsa scale


def _dma_r(nc, dst_ap, src_ap):
    nc.sync.dma_start(out=dst_ap, in_=src_ap.bitcast(F32R))


class Ctx:
    pass


def _declare_params(nc):
    P = Ctx()
    dp = nc.declare_dram_parameter
    P.xfm = dp("xfm", [NB, D, NX], F32, isOutput=False)
    P.yfm = dp("yfm", [NB, D, NYP], F32, isOutput=False)
    P.xrm = dp("xrm", [NB, NX, D], F32, isOutput=False)
    P.yrm = dp("yrm", [NB, NYR, D], F32, isOutput=False)
    P.latfm = dp("latfm", [D, L], F32, isOutput=False)
    P.ident = dp("ident", [128, 128], F32, isOutput=False)
    P.ones = dp("ones", [128, 128], F32, isOutput=False)
    P.sa = dp("sa", [1, 1], F32, isOutput=False)
    P.sv = dp("sv", [1, 1], F32, isOutput=False)
    for p in ("s", "r"):
        for nm, shp in (
            ("qkvb", [128, 18]), ("projb", [128, 6]),
            ("fc1b", [128, 24]), ("fc2b", [128, 6]),
            ("downb", [ADIM, 1]), ("downb2", [ADIM, 1]),
            ("ln1g", [128, 6]), ("ln1b", [128, 6]),
            ("ln2g", [128, 6]), ("ln2b", [128, 6]),
        ):
            setattr(P, f"{p}_{nm}", dp(f"{p}_{nm}", shp, F32, isOutput=False))
        for nm, shp in (
            ("qkvwT", [D, 3 * D]), ("projwT", [D, D]),
            ("fc1wT", [D, DFF]), ("fc2wT", [DFF, D]),
            ("downwT", [D, ADIM]), ("upwT", [ADIM, D]),
        ):
            setattr(P, f"{p}_{nm}", dp(f"{p}_{nm}", shp, BF16, isOutput=False))
    P.xout = dp("xout", [NB, D, NX], F32, isOutput=True)
    P.yout = dp("yout", [NB, D, NYR], F32, isOutput=True)
    return P


def build(stage="full"):
    nc = bacc.Bacc(None, target_bir_lowering=False)
    P = _declare_params(nc)
    alp = nc.allow_low_precision("f32r rounding is intended")
    alp.__enter__()

    with tile.TileContext(nc) as tc:
        _build_body(nc, tc, P, stage)
    alp.__exit__(None, None, None)
    nc.finalize()
    return nc


def _build_body(nc, tc, P, stage):
    from contextlib import ExitStack

    ctx = ExitStack()
    const = ctx.enter_context(tc.tile_pool(name="const", bufs=1))
    act = ctx.enter_context(tc.tile_pool(name="act", bufs=1))
    tmp = ctx.enter_context(tc.tile_pool(name="tmp", bufs=2))
    pg = ctx.enter_context(tc.tile_pool(name="pg", bufs=4, space="PSUM"))
    po = ctx.enter_context(tc.tile_pool(name="po", bufs=2, space="PSUM"))
    pst = ctx.enter_context(tc.tile_pool(name="pst", bufs=2, space="PSUM"))

    # ---- constants ----
    ident = const.tile([128, 128], F32, name="ident")
    nc.sync.dma_start(out=ident[:], in_=P.ident[:, :])
    ones = const.tile([128, 128], F32R, name="ones")
    _dma_r(nc, ones[:], P.ones[:, :])
    sa_t = const.tile([1, 1], F32, name="sa_t")
    nc.sync.dma_start(out=sa_t[:], in_=P.sa[:, :])
    sv_t = const.tile([1, 1], F32, name="sv_t")
    nc.sync.dma_start(out=sv_t[:], in_=P.sv[:, :])
    eps_t = const.tile([1, 1], F32, name="eps_t")
    nc.vector.memset(eps_t[:], EPS)

    bias = {}
    for p in ("s", "r"):
        for nm in ("qkvb", "projb", "fc1b", "fc2b", "ln1g", "ln1b",
                   "ln2g", "ln2b"):
            h = getattr(P, f"{p}_{nm}")
            t = const.tile(list(h.shape), F32, name=f"{p}_{nm}_t")
            nc.sync.dma_start(out=t[:], in_=h[:, :])
            bias[f"{p}_{nm}"] = t
        for nm in ("downb", "downb2"):
            h = getattr(P, f"{p}_{nm}")
            t = const.tile([ADIM, 1], F32, name=f"{p}_{nm}_t")
            nc.sync.dma_start(out=t[:], in_=h[:, :])
            bias[f"{p}_{nm}"] = t

    lat = [const.tile([128, L], F32R, name=f"lat{k}") for k in range(KC)]
    for k in range(KC):
        _dma_r(nc, lat[k][:], P.latfm[k * 128:(k + 1) * 128, :])

    # ---- stream state tiles (xfm / yfm hold the running residual) ----
    xfm = [[act.tile([128, NX], F32R, name=f"xfm{b}_{k}")
            for k in range(KC)] for b in range(NB)]
    yfm = [act.tile([128, 2 * NYP], F32R, name=f"yfm_{k}")
           for k in range(KC)]
    for b in range(NB):
        for k in range(KC):
            _dma_r(nc, xfm[b][k][:], P.xfm[b, k * 128:(k + 1) * 128, :])
            _dma_r(nc, yfm[k][:, b * NYP:(b + 1) * NYP],
                   P.yfm[b, k * 128:(k + 1) * 128, :])

    # V' ring: [128, 65] tiles, col 64 = ones (written once)
    vring = [const.tile([128, DH + 1], F32R, name=f"vr{i}") for i in range(5)]
    for i in range(5):
        _dma_r(nc, vring[i][:, DH:DH + 1], P.ones[:, 0:1])
    vslot = [0]

    # ================= FUSION =================
    with ExitStack() as fctx:
        fus = fctx.enter_context(tc.tile_pool(name="fus", bufs=1))
        rmp = fctx.enter_context(tc.tile_pool(name="rmp", bufs=1))
        ftmp = fctx.enter_context(tc.tile_pool(name="ftmp", bufs=2))

        # row-major copies of x/y for the P1^T @ concat_rm matmul
        xrm = [[rmp.tile([128, D], F32R, name=f"xrm{b}_{r}")
                for r in range(4)] for b in range(NB)]
        yrm = [[rmp.tile([128, D], F32R, name=f"yrm{b}_{r}")
                for r in range(2)] for b in range(NB)]
        for b in range(NB):
            for r in range(4):
                _dma_r(nc, xrm[b][r][:], P.xrm[b, r * 128:(r + 1) * 128, :])
            _dma_r(nc, yrm[b][0][:], P.yrm[b, 0:128, :])
            _dma_r(nc, yrm[b][1][0:68, :], P.yrm[b, 128:196, :])

        # --- S1 = lat @ concat^T, exp, per-batch softmax sums ---
        es1 = fus.tile([L, 1536], F32R, name="es1")
        sums = [[fus.tile([L, 1], F32, name=f"s1s{b}_{i}") for i in range(2)]
                for b in range(NB)]
        for b in range(NB):
            s1x = pg.tile([L, NX], F32, name="s1x", tag="pg")
            for k in range(KC):
                nc.tensor.matmul(s1x[:], lat[k][:], xfm[b][k][:],
                                 start=(k == 0), stop=(k == KC - 1))
            nc.scalar.activation(es1[:, b * NX:(b + 1) * NX], s1x[:], AF.Exp,
                                 scale=RS2, accum_out=sums[b][0][:])
            s1y = pg.tile([L, NYP], F32, name="s1y", tag="pg")
            for k in range(KC):
                nc.tensor.matmul(s1y[:], lat[k][:],
                                 yfm[k][:, b * NYP:(b + 1) * NYP],
                                 start=(k == 0), stop=(k == KC - 1))
            yo = 1024 + b * NYP
            nc.scalar.activation(es1[:, yo:yo + NYR], s1y[:, 0:NYR], AF.Exp,
                                 scale=RS2, accum_out=sums[b][1][:])

        # P1 = exp/sum (per batch), in place
        for b in range(NB):
            st = fus.tile([L, 1], F32, name=f"s1t{b}")
            nc.vector.tensor_tensor(st[:], sums[b][0][:], sums[b][1][:], OP.add)
            rec = fus.tile([L, 1], F32, name=f"s1r{b}")
            nc.vector.reciprocal(rec[:], st[:])
            nc.vector.tensor_scalar(
                es1[:, b * NX:(b + 1) * NX], es1[:, b * NX:(b + 1) * NX],
                rec[:], None, OP.mult)
            yo = 1024 + b * NYP
            nc.vector.tensor_scalar(
                es1[:, yo:yo + NYR], es1[:, yo:yo + NYR],
                rec[:], None, OP.mult)

        # --- fused = P1 @ concat  (row-major [L, 768] per batch) ---
        fusrm = [fus.tile([L, D], F32R, name=f"fusrm{b}") for b in range(NB)]
        for b in range(NB):
            # P1^T chunks: x: 4x[rows=128], y: [128],[68]
            p1t = []
            chunks = [(b * NX + c * 128, 128, xrm[b][c]) for c in range(4)]
            yo = 1024 + b * NYP
            chunks.append((yo, 128, yrm[b][0]))
            chunks.append((yo + 128, 68, yrm[b][1]))
            for ci, (off, cnt, _) in enumerate(chunks):
                tp = pg.tile([128, L], F32, name="p1tp", tag="pg")
                nc.tensor.matmul(tp[0:cnt, :],
                                 es1[:, off:off + cnt].bitcast(F32),
                                 ident[0:L, 0:L], is_transpose=True)
                t = ftmp.tile([128, L], F32R, name="p1t", tag="p1t", bufs=6)
                nc.scalar.activation(t[0:cnt, :], tp[0:cnt, :], AF.Identity)
                p1t.append(t)
            for dc, dw in ((0, 512), (512, 256)):
                fps = pg.tile([L, dw], F32, name="fps", tag="pg")
                for ci, (off, cnt, rm) in enumerate(chunks):
                    nc.tensor.matmul(fps[:], p1t[ci][0:cnt, :],
                                     rm[0:cnt, dc:dc + dw],
                                     start=(ci == 0), stop=(ci == len(chunks) - 1))
                nc.scalar.activation(fusrm[b][:, dc:dc + dw], fps[:],
                                     AF.Identity)

        # fused feature-major [D, L]
        fusfm = [[fus.tile([128, L], F32R, name=f"fusfm{b}_{k}")
                  for k in range(KC)] for b in range(NB)]
        for b in range(NB):
            for k in range(KC):
                tp = pg.tile([128, L], F32, name="ffmp", tag="pg")
                nc.tensor.matmul(tp[:], fusrm[b][:, k * 128:(k + 1) * 128].bitcast(F32),
                                 ident[0:L, 0:L], is_transpose=True)
                nc.scalar.activation(fusfm[b][k][:], tp[:], AF.Identity)

        # --- x += sa * shattn(x, fused, fused);  y += sv * shattn(y, ...) ---
        def latent_attn(fm_aps, ncols, scale_t, b):
            sps = pg.tile([L, ncols], F32, name="sps", tag="pg")
            for k in range(KC):
                nc.tensor.matmul(sps[:], fusfm[b][k][:], fm_aps[k],
                                 start=(k == 0), stop=(k == KC - 1))
            es = ftmp.tile([L, ncols], F32R, name="es23", tag="es23")
            nc.scalar.activation(es[:], sps[:], AF.Exp, scale=RS2)
            ssum = pst.tile([1, ncols], F32, name="ssum", tag="pst")
            nc.tensor.matmul(ssum[:], ones[0:L, 0:1], es[:],
                             start=True, stop=True)
            recf = ftmp.tile([1, ncols], F32, name="recf", tag="recf")
            nc.vector.reciprocal_approx_fast(recf[:], ssum[:])
            rec = ftmp.tile([1, ncols], F32R, name="rec23", tag="rec23")
            nc.vector.tensor_scalar(rec[:], recf[:], scale_t[:], None, OP.mult)
            rbp = pg.tile([128, ncols], F32, name="rbp", tag="pg")
            nc.tensor.matmul(rbp[:], ones[0:1, :], rec[:], start=True, stop=True)
            rb = ftmp.tile([128, ncols], F32, name="rb23", tag="rb23")
            nc.scalar.activation(rb[:], rbp[:], AF.Identity)
            for k in range(KC):
                ops = pg.tile([128, ncols], F32, name="ops", tag="pg")
                nc.tensor.matmul(ops[:], fusrm[b][:, k * 128:(k + 1) * 128],
                                 es[:], start=True, stop=True)
                tv = ftmp.tile([128, ncols], F32, name="tv23", tag="tv23")
                nc.vector.tensor_tensor(tv[:], ops[:], rb[:], OP.mult)
                nc.vector.tensor_tensor(fm_aps[k], fm_aps[k].bitcast(F32),
                                        tv[:], OP.add)

        for b in range(NB):
            latent_attn([xfm[b][k][:] for k in range(KC)], NX, sa_t, b)
            latent_attn([yfm[k][:, b * NYP:(b + 1) * NYP] for k in range(KC)],
                        NYP, sv_t, b)

    if stage == "fusion":
        for b in range(NB):
            for k in range(KC):
                nc.sync.dma_start(out=P.xout[b, k * 128:(k + 1) * 128, :],
                                  in_=xfm[b][k][:].bitcast(F32))
                nc.sync.dma_start(
                    out=P.yout[b, k * 128:(k + 1) * 128, :],
                    in_=yfm[k][:, b * NYP:b * NYP + NYR].bitcast(F32))
        ctx.close()
        return

    # ================= STREAMS =================
    wpool = ctx.enter_context(tc.tile_pool(name="wp", bufs=9))
    hpool = ctx.enter_context(tc.tile_pool(name="hp", bufs=12))
    qkvp = ctx.enter_context(tc.tile_pool(name="qkvp", bufs=18))
    mlpp = ctx.enter_context(tc.tile_pool(name="mlpp", bufs=26))
    ofmp = ctx.enter_context(tc.tile_pool(name="ofmp", bufs=8))

    def layernorm(chunks, g_t, b_t, out_dt=BF16):
        """chunks: list of (tiles, ncols). returns list of h tile lists."""
        out = []
        for tiles, ncols in chunks:
            mps = pst.tile([1, ncols], F32, name="mps", tag="pst")
            for k in range(KC):
                nc.tensor.matmul(mps[:], ones[:, 0:1], tiles[k][:],
                                 start=(k == 0), stop=(k == KC - 1))
            s2ps = pst.tile([1, ncols], F32, name="s2ps", tag="pst")
            for k in range(KC):
                sq = tmp.tile([128, ncols], F32R, name="sq", tag="sq", bufs=1)
                nc.vector.tensor_tensor(sq[:], tiles[k][:].bitcast(F32),
                                        tiles[k][:].bitcast(F32), OP.mult)
                nc.tensor.matmul(s2ps[:], ones[:, 0:1], sq[:],
                                 start=(k == 0), stop=(k == KC - 1))
            mb = tmp.tile([1, ncols], F32R, name="mb", tag="lnt1")
            t2 = tmp.tile([1, ncols], F32, name="t2", tag="lnt2")
            sdt = tmp.tile([1, ncols], F32R, name="sdt", tag="lnsd")
            nc.vector.tensor_scalar(mb[:], mps[:], 1.0 / D, None, OP.mult)
            nc.vector.tensor_tensor(t2[:], mb[:].bitcast(F32),
                                    mb[:].bitcast(F32), OP.mult)
            nc.vector.scalar_tensor_tensor(t2[:], s2ps[:], 1.0 / D, t2[:],
                                           OP.mult, OP.subtract)
            nc.scalar.activation(sdt[:], t2[:], AF.Sqrt, bias=eps_t[:])
            Mps = pg.tile([128, ncols], F32, name="Mps", tag="pg")
            nc.tensor.matmul(Mps[:], ones[0:1, :], mb[:], start=True, stop=True)
            Sps = pg.tile([128, ncols], F32, name="Sps", tag="pg")
            nc.tensor.matmul(Sps[:], ones[0:1, :], sdt[:], start=True, stop=True)
            Ab = tmp.tile([128, ncols], F32, name="Ab", tag="lnAb", bufs=2)
            nc.vector.reciprocal_approx_fast(Ab[:], Sps[:])
            hts = []
            for k in range(KC):
                z = tmp.tile([128, ncols], F32, name="z", tag="z", bufs=1)
                nc.vector.tensor_tensor(z[:], tiles[k][:].bitcast(F32), Mps[:],
                                        OP.subtract)
                nc.vector.tensor_tensor(z[:], z[:], Ab[:], OP.mult)
                ht = hpool.tile([128, ncols], out_dt, name="ht", tag="ht")
                nc.vector.tensor_scalar(ht[:], z[:], g_t[:, k:k + 1],
                                        b_t[:, k:k + 1], OP.mult, OP.add)
                hts.append(ht)
            out.append(hts)
        return out

    def gemm(wT, F, src_chunks, evict, wtag="ws"):
        """out[f, n] = sum_k wT[k, f] * src[k, n]; evict(ps, fc, ci).
        Weights stream in [128, 512]-wide tiles (4 f-chunks per DMA).
        src_chunks entries: (tiles, dn, vw) with vw mapping a [128, nc]
        tile AP to the dense moving-operand view of free size dn."""
        for fb in range(0, F, 512):
            fbw = min(512, F - fb)
            wts = []
            for k in range(KC):
                wt = wpool.tile([128, 512], BF16, name=f"w{fb}_{k}", tag=wtag)
                nc.sync.dma_start(out=wt[:, 0:fbw],
                                  in_=wT[k * 128:(k + 1) * 128, fb:fb + fbw])
                wts.append(wt)
            for fs in range(0, fbw, 128):
                fc = (fb + fs) // 128
                for ci, (tiles, dn, vw) in enumerate(src_chunks):
                    ps = pg.tile([128, dn], F32, name="gps", tag="pg")
                    for k in range(KC):
                        nc.tensor.matmul(ps[:], wts[k][:, fs:fs + 128],
                                         vw(tiles[k][:]),
                                         start=(k == 0), stop=(k == KC - 1))
                    evict(ps, fc, ci)

    def attention(qkv, ofm, batches):
        for h in range(H):
            po_, pslc = h // 2, slice((h % 2) * 64, (h % 2) * 64 + 64)
            pb = (h % 2) * 64
            for (boff, ncb, nreal) in batches:
                nkr = [(boff + c * 128, min(128, nreal - c * 128))
                       for c in range((nreal + 127) // 128)]
                q_ap = qkv[po_][pslc, boff:boff + ncb]
                vps = []
                for (off, cnt) in nkr:
                    vp = vring[vslot[0] % 5]
                    vslot[0] += 1
                    tp = pg.tile([128, DH], F32, name="vtp", tag="pg")
                    nc.tensor.matmul(
                        tp[0:cnt, :],
                        qkv[12 + po_][pslc, off:off + cnt].bitcast(F32),
                        ident[pb:pb + DH, pb:pb + DH], is_transpose=True)
                    nc.vector.tensor_copy(vp[0:cnt, 0:DH], tp[0:cnt, :])
                    vps.append(vp)
                ops = po.tile([DH + 1, ncb], F32, name="attops", tag="po")
                for ci, (off, cnt) in enumerate(nkr):
                    sps = pg.tile([128, ncb], F32, name="attsps", tag="pg")
                    nc.tensor.matmul(sps[0:cnt, :],
                                     qkv[6 + po_][pslc, off:off + cnt],
                                     q_ap, start=True, stop=True)
                    es = tmp.tile([128, ncb], F32R, name="attes", tag="attes", bufs=4)
                    nc.scalar.activation(es[0:cnt, :], sps[0:cnt, :], AF.Exp,
                                         scale=RS8)
                    nc.tensor.matmul(ops[:], vps[ci][0:cnt, :], es[0:cnt, :],
                                     start=(ci == 0), stop=(ci == len(nkr) - 1))
                ssb = tmp.tile([1, ncb], F32R, name="attssb", tag="attrec",
                               bufs=1)
                nc.scalar.activation(ssb[:], ops[DH:DH + 1, :], AF.Identity)
                rbp = pg.tile([DH, ncb], F32, name="attrbp", tag="pg")
                nc.tensor.matmul(rbp[:], ones[0:1, 0:DH], ssb[:],
                                 start=True, stop=True)
                rb = tmp.tile([DH, ncb], F32, name="attrb", tag="attrb",
                              bufs=1)
                nc.vector.reciprocal_approx_fast(rb[:], rbp[:])
                nc.vector.tensor_tensor(ofm[po_][pslc, boff:boff + ncb],
                                        ops[0:DH, :], rb[:], OP.mult)

    def stream(pfx, chunks, upto="full"):
        lnchunks = [(ch["st"], ch["nc"]) for ch in chunks]
        h1 = layernorm(lnchunks, bias[f"{pfx}_ln1g"], bias[f"{pfx}_ln1b"])
        wT = getattr(P, f"{pfx}_qkvwT")
        qb = bias[f"{pfx}_qkvb"]
        for ci_, ch in enumerate(chunks):
            ncols = ch["nc"]
            dn = ch.get("dn", ncols)
            vw = ch.get("vw") or (lambda a: a)
            pv = ch.get("pv") or (lambda a: a)
            qkv = [qkvp.tile([128, ncols], F32R, name=f"qkv{ci_}_{j}",
                             tag="qkv") for j in range(18)]

            def ev_qkv(ps, fc, ci, qkv=qkv, qb=qb, vw=vw, pv=pv):
                nc.vector.tensor_scalar(vw(qkv[fc][:]), pv(ps[:]),
                                        qb[:, fc:fc + 1], None, OP.add)

            gemm(wT, 3 * D, [(h1[ci_], dn, vw)], ev_qkv)
            ofm = [ofmp.tile([128, ncols], BF16, name=f"ofm{ci_}_{k}",
                             tag="ofm") for k in range(KC)]
            attention(qkv, ofm, ch["bt"])

            def ev_proj(ps, fc, ci, ch=ch, vw=vw, pv=pv):
                nc.vector.scalar_tensor_tensor(
                    vw(ch["st"][fc][:]), pv(ps[:]),
                    bias[f"{pfx}_projb"][:, fc:fc + 1],
                    vw(ch["st"][fc][:].bitcast(F32)), OP.add, OP.add)

            gemm(getattr(P, f"{pfx}_projwT"), D, [(ofm, dn, vw)], ev_proj)
        if upto == "attn":
            return
        h2 = layernorm(lnchunks, bias[f"{pfx}_ln2g"], bias[f"{pfx}_ln2b"])
        upT = getattr(P, f"{pfx}_upwT")
        for ci_, ch in enumerate(chunks):
            ncols = ch["nc"]
            dn = ch.get("dn", ncols)
            vw = ch.get("vw") or (lambda a: a)
            pv = ch.get("pv") or (lambda a: a)
            aps = pg.tile([ADIM, dn], F32, name="aps", tag="pg")
            dwT = getattr(P, f"{pfx}_downwT")
            for k in range(KC):
                dwt = wpool.tile([128, ADIM], BF16, name=f"dw{k}", tag="ws")
                nc.sync.dma_start(out=dwt[:], in_=dwT[k * 128:(k + 1) * 128, :])
                nc.tensor.matmul(aps[:], dwt[:], vw(h2[ci_][k][:]),
                                 start=(k == 0), stop=(k == KC - 1))
            eh = tmp.tile([ADIM, dn], F32, name="eh", tag="eh", bufs=1)
            nc.scalar.activation(eh[:], aps[:], AF.Exp, scale=-1.702,
                                 bias=bias[f"{pfx}_downb"][:])
            nc.vector.tensor_scalar(eh[:], eh[:], 1.0, None, OP.add)
            sig = tmp.tile([ADIM, dn], F32, name="sig", tag="sig", bufs=1)
            nc.vector.reciprocal_approx_fast(sig[:], eh[:])
            ah = tmp.tile([ADIM, dn], BF16, name="ah", tag="ah", bufs=1)
            nc.vector.scalar_tensor_tensor(ah[:], aps[:],
                                           bias[f"{pfx}_downb2"][:],
                                           sig[:], OP.add, OP.mult)
            mlp = [mlpp.tile([128, ncols], BF16, name=f"mlp{ci_}_{j}",
                             tag="mlp") for j in range(24)]

            def ev_fc1(ps, fc, ci, mlp=mlp, vw=vw, pv=pv):
                nc.scalar.activation(vw(mlp[fc][:]), pv(ps[:]), AF.Gelu,
                                     bias=bias[f"{pfx}_fc1b"][:, fc:fc + 1])

            gemm(getattr(P, f"{pfx}_fc1wT"), DFF, [(h2[ci_], dn, vw)], ev_fc1)
            f2T = getattr(P, f"{pfx}_fc2wT")
            for fc in range(KC):
                ps = pg.tile([128, dn], F32, name="f2ps", tag="pg")
                for kb in range(6):
                    wt = wpool.tile([128, 4, 128], BF16, name=f"f2w{kb}",
                                    tag="ws")
                    src = f2T[kb * 512:(kb + 1) * 512,
                              fc * 128:(fc + 1) * 128]
                    nc.sync.dma_start(
                        out=wt[:], in_=src.rearrange("(n p) m -> p n m", p=128))
                    for j in range(4):
                        k = kb * 4 + j
                        nc.tensor.matmul(ps[:], wt[:, j, :], vw(mlp[k][:]),
                                         start=(k == 0), stop=False)
                uwt = wpool.tile([ADIM, 128], BF16, name=f"uw{fc}", tag="ws")
                nc.sync.dma_start(out=uwt[:],
                                  in_=upT[:, fc * 128:(fc + 1) * 128])
                nc.tensor.matmul(ps[:], uwt[:], ah[:], start=False, stop=True)
                nc.vector.scalar_tensor_tensor(
                    vw(ch["st"][fc][:]), pv(ps[:]),
                    bias[f"{pfx}_fc2b"][:, fc:fc + 1],
                    vw(ch["st"][fc][:].bitcast(F32)), OP.add, OP.add)

    upto = "attn" if stage == "attn" else "full"
    xchunks = [{"st": xfm[b], "nc": NX, "bt": [(0, NX, NX)]}
               for b in range(NB)]
    def _yvw(a):
        return a.rearrange("p (b c) -> p b c", b=2)[:, :, 0:NYR]

    def _ypv(a):
        return a.rearrange("p (b c) -> p b c", b=2)

    ychunks = [{"st": yfm, "nc": 2 * NYP, "dn": 2 * NYR,
                "vw": _yvw, "pv": _ypv,
                "bt": [(0, NYP, NYR), (NYP, NYP, NYR)]}]
    stream("s", xchunks, upto)
    for b in range(NB):
        for k in range(KC):
            nc.sync.dma_start(out=P.xout[b, k * 128:(k + 1) * 128, :],
                              in_=xfm[b][k][:].bitcast(F32))
    stream("r", ychunks, upto)
    for b in range(NB):
        for k in range(KC):
            nc.sync.dma_start(
                out=P.yout[b, k * 128:(k + 1) * 128, :],
                in_=yfm[k][:, b * NYP:b * NYP + NYR].bitcast(F32))
    ctx.close()


# ============================ host side ============================

_CACHE = {}


def _get_nc(stage="full"):
    if stage not in _CACHE:
        _CACHE[stage] = build(stage)
    return _CACHE[stage]


def _prep_core_inputs(c, x, y, latents, scale_a, scale_v, W):
    b0 = 2 * c
    xb = x[b0:b0 + 2]
    yb = y[b0:b0 + 2]
    yfm = np.zeros((NB, D, NYP), np.float32)
    yfm[:, :, :NYR] = yb.transpose(0, 2, 1)
    m = {
        "xfm": np.ascontiguousarray(xb.transpose(0, 2, 1)),
        "yfm": yfm,
        "xrm": np.ascontiguousarray(xb),
        "yrm": np.ascontiguousarray(yb),
    }
    m.update(W)
    return m


def _shared_inputs(latents, scale_a, scale_v, kw):
    W = {
        "latfm": np.ascontiguousarray(latents[0].T),
        "ident": np.eye(128, dtype=np.float32),
        "ones": np.ones((128, 128), np.float32),
        "sa": np.array([[float(scale_a[0])]], np.float32),
        "sv": np.array([[float(scale_v[0])]], np.float32),
    }
    for p, q in (("s", "spec"), ("r", "rgb")):
        g = lambda nm: np.asarray(kw[f"{q}_{nm}"], np.float32)
        sc = float(g("scale")[0])
        import ml_dtypes
        bf = ml_dtypes.bfloat16
        W[f"{p}_qkvwT"] = np.ascontiguousarray(g("qkv_w").T.astype(bf))
        W[f"{p}_projwT"] = np.ascontiguousarray(g("proj_w").T.astype(bf))
        W[f"{p}_fc1wT"] = np.ascontiguousarray(g("fc1_w").T.astype(bf))
        W[f"{p}_fc2wT"] = np.ascontiguousarray(g("fc2_w").T.astype(bf))
        W[f"{p}_downwT"] = np.ascontiguousarray(g("down_w").T.astype(bf))
        W[f"{p}_upwT"] = np.ascontiguousarray((g("up_w").T * sc).astype(bf))
        W[f"{p}_qkvb"] = np.ascontiguousarray(g("qkv_b").reshape(18, 128).T)
        W[f"{p}_projb"] = np.ascontiguousarray(g("proj_b").reshape(6, 128).T)
        W[f"{p}_fc1b"] = np.ascontiguousarray(g("fc1_b").reshape(24, 128).T)
        W[f"{p}_fc2b"] = np.ascontiguousarray(
            (g("fc2_b") + g("up_b") * sc).reshape(6, 128).T)
        W[f"{p}_downb"] = np.ascontiguousarray(
            (g("down_b") * -1.702).reshape(ADIM, 1))
        W[f"{p}_downb2"] = np.ascontiguousarray(g("down_b").reshape(ADIM, 1))
        W[f"{p}_ln1g"] = np.ascontiguousarray(g("ln1_g").reshape(6, 128).T)
        W[f"{p}_ln1b"] = np.ascontiguousarray(g("ln1_b").reshape(6, 128).T)
        W[f"{p}_ln2g"] = np.ascontiguousarray(g("ln2_g").reshape(6, 128).T)
        W[f"{p}_ln2b"] = np.ascontiguousarray(g("ln2_b").reshape(6, 128).T)
    return W


def run(stage="full", trace=False, **kw):
    x = np.asarray(kw["x"], np.float32)
    y = np.asarray(kw["y"], np.float32)
    W = _shared_inputs(np.asarray(kw["latents"], np.float32),
                       np.asarray(kw["scale_a"], np.float32),
                       np.asarray(kw["scale_v"], np.float32), kw)
    in_maps = [_prep_core_inputs(c, x, y, None, None, None, W)
               for c in range(8)]
    nc = _get_nc(stage)
    res = run_bass_kernel_spmd(nc, in_maps, core_ids=list(range(8)),
                               trace=trace)
    xs, ys = [], []
    for c in range(8):
        r = res.results[c]
        xs.append(r["xout"].transpose(0, 2, 1))
        ys.append(r["yout"].transpose(0, 2, 1))
    xo = np.concatenate(xs, axis=0)
    yo = np.concatenate(ys, axis=0)
    return (xo, yo), res


def kernel(**inputs):
    out, _ = run(stage="full", trace=False, **inputs)
    return out


# revision 45
# speedup vs baseline: 1.2143x; 1.1779x over previous
"""AdaptFormer fused kernel for one TRN2 chip (8 NeuronCores).

Strategy: pure data-parallel over batch (16 batches -> 2 per core). Each core
runs the full fusion + both transformer streams for its 2 batches. All
activations live feature-major ([D on partitions, tokens on free dim]) so every
GEMM runs weights-stationary with zero transposes; LayerNorm/softmax partition
reductions are done with ones-vector matmuls on the TensorEngine, and
[1,N]->[P,N] broadcasts with ones-stationary matmuls. Matmul dtype is float32r
(TF32): full throughput at free-dim >= 256 with ~1e-4 relative rounding.
"""

import sys

sys.path.insert(0, "/opt/trn_rl_repo")

import math
import numpy as np

import concourse.bass as bass
import concourse.bacc as bacc
import concourse.mybir as mybir
import concourse.tile as tile
from concourse.bass_utils import run_bass_kernel_spmd
from concourse.masks import make_identity

F32 = mybir.dt.float32
F32R = mybir.dt.float32r
BF16 = mybir.dt.bfloat16
AF = mybir.ActivationFunctionType
OP = mybir.AluOpType

D = 768
KC = 6          # D / 128
H = 12
DH = 64
L = 32
DFF = 3072
ADIM = 64
EPS = 1e-6
NB = 2          # batches per core
NX = 512        # x tokens per batch
NYR = 196       # real y tokens per batch
NYP = 256       # padded y tokens per batch
RS2 = 1.0 / math.sqrt(D)   # fusion attention scale
RS8 = 1.0 / math.sqrt(DH)  # mhsa scale


def _dma_r(nc, dst_ap, src_ap):
    nc.sync.dma_start(out=dst_ap, in_=src_ap.bitcast(F32R))


class Ctx:
    pass


def _declare_params(nc):
    P = Ctx()
    dp = nc.declare_dram_parameter
    P.xfm = dp("xfm", [NB, D, NX], F32, isOutput=False)
    P.yfm = dp("yfm", [NB, D, NYP], F32, isOutput=False)
    P.xrm = dp("xrm", [NB, NX, D], F32, isOutput=False)
    P.yrm = dp("yrm", [NB, NYR, D], F32, isOutput=False)
    P.latfm = dp("latfm", [D, L], F32, isOutput=False)
    P.ident = dp("ident", [128, 128], F32, isOutput=False)
    P.ones = dp("ones", [128, 128], F32, isOutput=False)
    P.sa = dp("sa", [1, 1], F32, isOutput=False)
    P.sv = dp("sv", [1, 1], F32, isOutput=False)
    for p in ("s", "r"):
        for nm, shp in (
            ("qkvb", [128, 18]), ("projb", [128, 6]),
            ("fc1b", [128, 24]), ("fc2b", [128, 6]),
            ("downb", [ADIM, 1]), ("downb2", [ADIM, 1]),
            ("ln1g", [128, 6]), ("ln1b", [128, 6]),
            ("ln2g", [128, 6]), ("ln2b", [128, 6]),
        ):
            setattr(P, f"{p}_{nm}", dp(f"{p}_{nm}", shp, F32, isOutput=False))
        for nm, shp in (
            ("qkvwT", [D, 3 * D]), ("projwT", [D, D]),
            ("fc1wT", [D, DFF]), ("fc2wT", [DFF, D]),
            ("downwT", [D, ADIM]), ("upwT", [ADIM, D]),
        ):
            setattr(P, f"{p}_{nm}", dp(f"{p}_{nm}", shp, BF16, isOutput=False))
    P.xout = dp("xout", [NB, D, NX], F32, isOutput=True)
    P.yout = dp("yout", [NB, D, NYR], F32, isOutput=True)
    return P


def build(stage="full"):
    nc = bacc.Bacc(None, target_bir_lowering=False)
    P = _declare_params(nc)
    alp = nc.allow_low_precision("f32r rounding is intended")
    alp.__enter__()

    with tile.TileContext(nc) as tc:
        _build_body(nc, tc, P, stage)
    alp.__exit__(None, None, None)
    nc.finalize()
    return nc


def _build_body(nc, tc, P, stage):
    from contextlib import ExitStack

    ctx = ExitStack()
    const = ctx.enter_context(tc.tile_pool(name="const", bufs=1))
    act = ctx.enter_context(tc.tile_pool(name="act", bufs=1))
    tmp = ctx.enter_context(tc.tile_pool(name="tmp", bufs=2))
    pg = ctx.enter_context(tc.tile_pool(name="pg", bufs=4, space="PSUM"))
    po = ctx.enter_context(tc.tile_pool(name="po", bufs=2, space="PSUM"))
    pst = ctx.enter_context(tc.tile_pool(name="pst", bufs=2, space="PSUM"))

    # ---- constants ----
    ident = const.tile([128, 128], F32, name="ident")
    nc.sync.dma_start(out=ident[:], in_=P.ident[:, :])
    ones = const.tile([128, 128], F32R, name="ones")
    _dma_r(nc, ones[:], P.ones[:, :])
    sa_t = const.tile([1, 1], F32, name="sa_t")
    nc.sync.dma_start(out=sa_t[:], in_=P.sa[:, :])
    sv_t = const.tile([1, 1], F32, name="sv_t")
    nc.sync.dma_start(out=sv_t[:], in_=P.sv[:, :])
    eps_t = const.tile([1, 1], F32, name="eps_t")
    nc.vector.memset(eps_t[:], EPS)

    bias = {}
    for p in ("s", "r"):
        for nm in ("qkvb", "projb", "fc1b", "fc2b", "ln1g", "ln1b",
                   "ln2g", "ln2b"):
            h = getattr(P, f"{p}_{nm}")
            t = const.tile(list(h.shape), F32, name=f"{p}_{nm}_t")
            nc.sync.dma_start(out=t[:], in_=h[:, :])
            bias[f"{p}_{nm}"] = t
        for nm in ("downb", "downb2"):
            h = getattr(P, f"{p}_{nm}")
            t = const.tile([ADIM, 1], F32, name=f"{p}_{nm}_t")
            nc.sync.dma_start(out=t[:], in_=h[:, :])
            bias[f"{p}_{nm}"] = t

    lat = [const.tile([128, L], F32R, name=f"lat{k}") for k in range(KC)]
    for k in range(KC):
        _dma_r(nc, lat[k][:], P.latfm[k * 128:(k + 1) * 128, :])

    # ---- stream state tiles (xfm / yfm hold the running residual) ----
    xfm = [[act.tile([128, NX], F32R, name=f"xfm{b}_{k}")
            for k in range(KC)] for b in range(NB)]
    yfm = [act.tile([128, 2 * NYP], F32R, name=f"yfm_{k}")
           for k in range(KC)]
    for b in range(NB):
        for k in range(KC):
            _dma_r(nc, xfm[b][k][:], P.xfm[b, k * 128:(k + 1) * 128, :])
            _dma_r(nc, yfm[k][:, b * NYP:(b + 1) * NYP],
                   P.yfm[b, k * 128:(k + 1) * 128, :])

    # V' ring: [128, 65] tiles, col 64 = ones (written once)
    vring = [const.tile([128, DH + 1], F32R, name=f"vr{i}") for i in range(5)]
    for i in range(5):
        _dma_r(nc, vring[i][:, DH:DH + 1], P.ones[:, 0:1])
    vslot = [0]

    # ================= FUSION =================
    with ExitStack() as fctx:
        fus = fctx.enter_context(tc.tile_pool(name="fus", bufs=1))
        rmp = fctx.enter_context(tc.tile_pool(name="rmp", bufs=1))
        ftmp = fctx.enter_context(tc.tile_pool(name="ftmp", bufs=2))

        # row-major copies of x/y for the P1^T @ concat_rm matmul
        xrm = [[rmp.tile([128, D], F32R, name=f"xrm{b}_{r}")
                for r in range(4)] for b in range(NB)]
        yrm = [[rmp.tile([128, D], F32R, name=f"yrm{b}_{r}")
                for r in range(2)] for b in range(NB)]
        for b in range(NB):
            for r in range(4):
                _dma_r(nc, xrm[b][r][:], P.xrm[b, r * 128:(r + 1) * 128, :])
            _dma_r(nc, yrm[b][0][:], P.yrm[b, 0:128, :])
            _dma_r(nc, yrm[b][1][0:68, :], P.yrm[b, 128:196, :])

        # --- S1 = lat @ concat^T, exp, per-batch softmax sums ---
        es1 = fus.tile([L, 1536], F32R, name="es1")
        sums = [[fus.tile([L, 1], F32, name=f"s1s{b}_{i}") for i in range(2)]
                for b in range(NB)]
        for b in range(NB):
            s1x = pg.tile([L, NX], F32, name="s1x", tag="pg")
            for k in range(KC):
                nc.tensor.matmul(s1x[:], lat[k][:], xfm[b][k][:],
                                 start=(k == 0), stop=(k == KC - 1))
            nc.scalar.activation(es1[:, b * NX:(b + 1) * NX], s1x[:], AF.Exp,
                                 scale=RS2, accum_out=sums[b][0][:])
            s1y = pg.tile([L, NYP], F32, name="s1y", tag="pg")
            for k in range(KC):
                nc.tensor.matmul(s1y[:], lat[k][:],
                                 yfm[k][:, b * NYP:(b + 1) * NYP],
                                 start=(k == 0), stop=(k == KC - 1))
            yo = 1024 + b * NYP
            nc.scalar.activation(es1[:, yo:yo + NYR], s1y[:, 0:NYR], AF.Exp,
                                 scale=RS2, accum_out=sums[b][1][:])

        # P1 = exp/sum (per batch), in place
        for b in range(NB):
            st = fus.tile([L, 1], F32, name=f"s1t{b}")
            nc.vector.tensor_tensor(st[:], sums[b][0][:], sums[b][1][:], OP.add)
            rec = fus.tile([L, 1], F32, name=f"s1r{b}")
            nc.vector.reciprocal(rec[:], st[:])
            nc.vector.tensor_scalar(
                es1[:, b * NX:(b + 1) * NX], es1[:, b * NX:(b + 1) * NX],
                rec[:], None, OP.mult)
            yo = 1024 + b * NYP
            nc.vector.tensor_scalar(
                es1[:, yo:yo + NYR], es1[:, yo:yo + NYR],
                rec[:], None, OP.mult)

        # --- fused = P1 @ concat  (row-major [L, 768] per batch) ---
        fusrm = [fus.tile([L, D], F32R, name=f"fusrm{b}") for b in range(NB)]
        for b in range(NB):
            # P1^T chunks: x: 4x[rows=128], y: [128],[68]
            p1t = []
            chunks = [(b * NX + c * 128, 128, xrm[b][c]) for c in range(4)]
            yo = 1024 + b * NYP
            chunks.append((yo, 128, yrm[b][0]))
            chunks.append((yo + 128, 68, yrm[b][1]))
            for ci, (off, cnt, _) in enumerate(chunks):
                tp = pg.tile([128, L], F32, name="p1tp", tag="pg")
                nc.tensor.matmul(tp[0:cnt, :],
                                 es1[:, off:off + cnt].bitcast(F32),
                                 ident[0:L, 0:L], is_transpose=True)
                t = ftmp.tile([128, L], F32R, name="p1t", tag="p1t", bufs=6)
                nc.scalar.activation(t[0:cnt, :], tp[0:cnt, :], AF.Identity)
                p1t.append(t)
            for dc, dw in ((0, 512), (512, 256)):
                fps = pg.tile([L, dw], F32, name="fps", tag="pg")
                for ci, (off, cnt, rm) in enumerate(chunks):
                    nc.tensor.matmul(fps[:], p1t[ci][0:cnt, :],
                                     rm[0:cnt, dc:dc + dw],
                                     start=(ci == 0), stop=(ci == len(chunks) - 1))
                nc.scalar.activation(fusrm[b][:, dc:dc + dw], fps[:],
                                     AF.Identity)

        # fused feature-major [D, L]
        fusfm = [[fus.tile([128, L], F32R, name=f"fusfm{b}_{k}")
                  for k in range(KC)] for b in range(NB)]
        for b in range(NB):
            for k in range(KC):
                tp = pg.tile([128, L], F32, name="ffmp", tag="pg")
                nc.tensor.matmul(tp[:], fusrm[b][:, k * 128:(k + 1) * 128].bitcast(F32),
                                 ident[0:L, 0:L], is_transpose=True)
                nc.scalar.activation(fusfm[b][k][:], tp[:], AF.Identity)

        # --- x += sa * shattn(x, fused, fused);  y += sv * shattn(y, ...) ---
        def latent_attn(fm_aps, ncols, scale_t, b):
            sps = pg.tile([L, ncols], F32, name="sps", tag="pg")
            for k in range(KC):
                nc.tensor.matmul(sps[:], fusfm[b][k][:], fm_aps[k],
                                 start=(k == 0), stop=(k == KC - 1))
            es = ftmp.tile([L, ncols], F32R, name="es23", tag="es23")
            nc.scalar.activation(es[:], sps[:], AF.Exp, scale=RS2)
            ssum = pst.tile([1, ncols], F32, name="ssum", tag="pst")
            nc.tensor.matmul(ssum[:], ones[0:L, 0:1], es[:],
                             start=True, stop=True)
            recf = ftmp.tile([1, ncols], F32, name="recf", tag="recf")
            nc.vector.reciprocal_approx_fast(recf[:], ssum[:])
            rec = ftmp.tile([1, ncols], F32R, name="rec23", tag="rec23")
            nc.vector.tensor_scalar(rec[:], recf[:], scale_t[:], None, OP.mult)
            rbp = pg.tile([128, ncols], F32, name="rbp", tag="pg")
            nc.tensor.matmul(rbp[:], ones[0:1, :], rec[:], start=True, stop=True)
            rb = ftmp.tile([128, ncols], F32, name="rb23", tag="rb23")
            nc.scalar.activation(rb[:], rbp[:], AF.Identity)
            for k in range(KC):
                ops = pg.tile([128, ncols], F32, name="ops", tag="pg")
                nc.tensor.matmul(ops[:], fusrm[b][:, k * 128:(k + 1) * 128],
                                 es[:], start=True, stop=True)
                tv = ftmp.tile([128, ncols], F32, name="tv23", tag="tv23")
                nc.vector.tensor_tensor(tv[:], ops[:], rb[:], OP.mult)
                nc.vector.tensor_tensor(fm_aps[k], fm_aps[k].bitcast(F32),
                                        tv[:], OP.add)

        for b in range(NB):
            latent_attn([xfm[b][k][:] for k in range(KC)], NX, sa_t, b)
            latent_attn([yfm[k][:, b * NYP:(b + 1) * NYP] for k in range(KC)],
                        NYP, sv_t, b)

    if stage == "fusion":
        for b in range(NB):
            for k in range(KC):
                nc.sync.dma_start(out=P.xout[b, k * 128:(k + 1) * 128, :],
                                  in_=xfm[b][k][:].bitcast(F32))
                nc.sync.dma_start(
                    out=P.yout[b, k * 128:(k + 1) * 128, :],
                    in_=yfm[k][:, b * NYP:b * NYP + NYR].bitcast(F32))
        ctx.close()
        return

    # ================= STREAMS =================
    wpool = ctx.enter_context(tc.tile_pool(name="wp", bufs=9))
    hpool = ctx.enter_context(tc.tile_pool(name="hp", bufs=12))
    qkvp = ctx.enter_context(tc.tile_pool(name="qkvp", bufs=18))
    mlpp = ctx.enter_context(tc.tile_pool(name="mlpp", bufs=26))
    ofmp = ctx.enter_context(tc.tile_pool(name="ofmp", bufs=8))

    def layernorm(chunks, g_t, b_t, out_dt=BF16):
        """chunks: list of chunk dicts. returns list of h tile lists."""
        out = []
        for ch in chunks:
            tiles, ncols = ch["st"], ch["nc"]
            dn = ch.get("dn", ncols)
            vw = ch.get("vw") or (lambda a: a)
            d3 = ch.get("pv") or (lambda a: a)
            mps = pst.tile([1, dn], F32, name="mps", tag="pst")
            for k in range(KC):
                nc.tensor.matmul(mps[:], ones[:, 0:1], vw(tiles[k][:]),
                                 start=(k == 0), stop=(k == KC - 1))
            s2ps = pst.tile([1, dn], F32, name="s2ps", tag="pst")
            for k in range(KC):
                sq = tmp.tile([128, dn], F32R, name="sq", tag="sq", bufs=1)
                nc.vector.tensor_tensor(d3(sq[:]),
                                        vw(tiles[k][:].bitcast(F32)),
                                        vw(tiles[k][:].bitcast(F32)), OP.mult)
                nc.tensor.matmul(s2ps[:], ones[:, 0:1], sq[:],
                                 start=(k == 0), stop=(k == KC - 1))
            mb = tmp.tile([1, dn], F32R, name="mb", tag="lnt1")
            t2 = tmp.tile([1, dn], F32, name="t2", tag="lnt2")
            sdt = tmp.tile([1, dn], F32R, name="sdt", tag="lnsd")
            nc.vector.tensor_scalar(mb[:], mps[:], 1.0 / D, None, OP.mult)
            nc.vector.tensor_tensor(t2[:], mb[:].bitcast(F32),
                                    mb[:].bitcast(F32), OP.mult)
            nc.vector.scalar_tensor_tensor(t2[:], s2ps[:], 1.0 / D, t2[:],
                                           OP.mult, OP.subtract)
            nc.scalar.activation(sdt[:], t2[:], AF.Sqrt, bias=eps_t[:])
            Mps = pg.tile([128, dn], F32, name="Mps", tag="pg")
            nc.tensor.matmul(Mps[:], ones[0:1, :], mb[:], start=True, stop=True)
            Sps = pg.tile([128, dn], F32, name="Sps", tag="pg")
            nc.tensor.matmul(Sps[:], ones[0:1, :], sdt[:], start=True, stop=True)
            Ab = tmp.tile([128, dn], F32, name="Ab", tag="lnAb", bufs=2)
            nc.vector.reciprocal_approx_fast(Ab[:], Sps[:])
            hts = []
            for k in range(KC):
                z = tmp.tile([128, dn], F32, name="z", tag="z", bufs=1)
                nc.vector.tensor_tensor(d3(z[:]),
                                        vw(tiles[k][:].bitcast(F32)),
                                        d3(Mps[:]), OP.subtract)
                nc.vector.tensor_tensor(z[:], z[:], Ab[:], OP.mult)
                ht = hpool.tile([128, ncols], out_dt, name="ht", tag="ht")
                nc.vector.tensor_scalar(vw(ht[:]), d3(z[:]), g_t[:, k:k + 1],
                                        b_t[:, k:k + 1], OP.mult, OP.add)
                hts.append(ht)
            out.append(hts)
        return out

    def gemm(wT, F, src_chunks, evict, wtag="ws"):
        """out[f, n] = sum_k wT[k, f] * src[k, n]; evict(ps, fc, ci).
        Weights stream in [128, 512]-wide tiles (4 f-chunks per DMA).
        src_chunks entries: (tiles, dn, vw) with vw mapping a [128, nc]
        tile AP to the dense moving-operand view of free size dn."""
        for fb in range(0, F, 512):
            fbw = min(512, F - fb)
            wts = []
            for k in range(KC):
                wt = wpool.tile([128, 512], BF16, name=f"w{fb}_{k}", tag=wtag)
                nc.sync.dma_start(out=wt[:, 0:fbw],
                                  in_=wT[k * 128:(k + 1) * 128, fb:fb + fbw])
                wts.append(wt)
            for fs in range(0, fbw, 128):
                fc = (fb + fs) // 128
                for ci, (tiles, dn, vw) in enumerate(src_chunks):
                    ps = pg.tile([128, dn], F32, name="gps", tag="pg")
                    for k in range(KC):
                        nc.tensor.matmul(ps[:], wts[k][:, fs:fs + 128],
                                         vw(tiles[k][:]),
                                         start=(k == 0), stop=(k == KC - 1))
                    evict(ps, fc, ci)

    def attention(qkv, ofm, batches):
        for h in range(H):
            po_, pslc = h // 2, slice((h % 2) * 64, (h % 2) * 64 + 64)
            pb = (h % 2) * 64
            for (boff, ncb, nreal) in batches:
                nkr = [(boff + c * 128, min(128, nreal - c * 128))
                       for c in range((nreal + 127) // 128)]
                q_ap = qkv[po_][pslc, boff:boff + ncb]
                vps = []
                for (off, cnt) in nkr:
                    vp = vring[vslot[0] % 5]
                    vslot[0] += 1
                    tp = pg.tile([128, DH], F32, name="vtp", tag="pg")
                    nc.tensor.matmul(
                        tp[0:cnt, :],
                        qkv[12 + po_][pslc, off:off + cnt].bitcast(F32),
                        ident[pb:pb + DH, pb:pb + DH], is_transpose=True)
                    nc.vector.tensor_copy(vp[0:cnt, 0:DH], tp[0:cnt, :])
                    vps.append(vp)
                ops = po.tile([DH + 1, ncb], F32, name="attops", tag="po")
                for ci, (off, cnt) in enumerate(nkr):
                    sps = pg.tile([128, ncb], F32, name="attsps", tag="pg")
                    nc.tensor.matmul(sps[0:cnt, :],
                                     qkv[6 + po_][pslc, off:off + cnt],
                                     q_ap, start=True, stop=True)
                    es = tmp.tile([128, ncb], F32R, name="attes", tag="attes", bufs=4)
                    nc.scalar.activation(es[0:cnt, :], sps[0:cnt, :], AF.Exp,
                                         scale=RS8)
                    nc.tensor.matmul(ops[:], vps[ci][0:cnt, :], es[0:cnt, :],
                                     start=(ci == 0), stop=(ci == len(nkr) - 1))
                ssb = tmp.tile([1, ncb], F32R, name="attssb", tag="attrec",
                               bufs=1)
                nc.scalar.activation(ssb[:], ops[DH:DH + 1, :], AF.Identity)
                rbp = pg.tile([DH, ncb], F32, name="attrbp", tag="pg")
                nc.tensor.matmul(rbp[:], ones[0:1, 0:DH], ssb[:],
                                 start=True, stop=True)
                rb = tmp.tile([DH, ncb], F32, name="attrb", tag="attrb",
                              bufs=1)
                nc.vector.reciprocal_approx_fast(rb[:], rbp[:])
                nc.vector.tensor_tensor(ofm[po_][pslc, boff:boff + ncb],
                                        ops[0:DH, :], rb[:], OP.mult)

    def stream(pfx, chunks, upto="full"):
        h1 = layernorm(chunks, bias[f"{pfx}_ln1g"], bias[f"{pfx}_ln1b"])
        wT = getattr(P, f"{pfx}_qkvwT")
        qb = bias[f"{pfx}_qkvb"]
        for ci_, ch in enumerate(chunks):
            ncols = ch["nc"]
            dn = ch.get("dn", ncols)
            vw = ch.get("vw") or (lambda a: a)
            pv = ch.get("pv") or (lambda a: a)
            qkv = [qkvp.tile([128, ncols], F32R, name=f"qkv{ci_}_{j}",
                             tag="qkv") for j in range(18)]

            def ev_qkv(ps, fc, ci, qkv=qkv, qb=qb, vw=vw, pv=pv):
                nc.vector.tensor_scalar(vw(qkv[fc][:]), pv(ps[:]),
                                        qb[:, fc:fc + 1], None, OP.add)

            gemm(wT, 3 * D, [(h1[ci_], dn, vw)], ev_qkv)
            ofm = [ofmp.tile([128, ncols], BF16, name=f"ofm{ci_}_{k}",
                             tag="ofm") for k in range(KC)]
            attention(qkv, ofm, ch["bt"])

            def ev_proj(ps, fc, ci, ch=ch, vw=vw, pv=pv):
                nc.vector.scalar_tensor_tensor(
                    vw(ch["st"][fc][:]), pv(ps[:]),
                    bias[f"{pfx}_projb"][:, fc:fc + 1],
                    vw(ch["st"][fc][:].bitcast(F32)), OP.add, OP.add)

            gemm(getattr(P, f"{pfx}_projwT"), D, [(ofm, dn, vw)], ev_proj)
        if upto == "attn":
            return
        h2 = layernorm(chunks, bias[f"{pfx}_ln2g"], bias[f"{pfx}_ln2b"])
        upT = getattr(P, f"{pfx}_upwT")
        for ci_, ch in enumerate(chunks):
            ncols = ch["nc"]
            dn = ch.get("dn", ncols)
            vw = ch.get("vw") or (lambda a: a)
            pv = ch.get("pv") or (lambda a: a)
            aps = pg.tile([ADIM, dn], F32, name="aps", tag="pg")
            dwT = getattr(P, f"{pfx}_downwT")
            for k in range(KC):
                dwt = wpool.tile([128, ADIM], BF16, name=f"dw{k}", tag="ws")
                nc.sync.dma_start(out=dwt[:], in_=dwT[k * 128:(k + 1) * 128, :])
                nc.tensor.matmul(aps[:], dwt[:], vw(h2[ci_][k][:]),
                                 start=(k == 0), stop=(k == KC - 1))
            eh = tmp.tile([ADIM, dn], F32, name="eh", tag="eh", bufs=1)
            nc.scalar.activation(eh[:], aps[:], AF.Exp, scale=-1.702,
                                 bias=bias[f"{pfx}_downb"][:])
            nc.vector.tensor_scalar(eh[:], eh[:], 1.0, None, OP.add)
            sig = tmp.tile([ADIM, dn], F32, name="sig", tag="sig", bufs=1)
            nc.vector.reciprocal_approx_fast(sig[:], eh[:])
            ah = tmp.tile([ADIM, dn], BF16, name="ah", tag="ah", bufs=1)
            nc.vector.scalar_tensor_tensor(ah[:], aps[:],
                                           bias[f"{pfx}_downb2"][:],
                                           sig[:], OP.add, OP.mult)
            mlp = [mlpp.tile([128, ncols], BF16, name=f"mlp{ci_}_{j}",
                             tag="mlp") for j in range(24)]

            def ev_fc1(ps, fc, ci, mlp=mlp, vw=vw, pv=pv):
                nc.scalar.activation(vw(mlp[fc][:]), pv(ps[:]), AF.Gelu,
                                     bias=bias[f"{pfx}_fc1b"][:, fc:fc + 1])

            gemm(getattr(P, f"{pfx}_fc1wT"), DFF, [(h2[ci_], dn, vw)], ev_fc1)
            f2T = getattr(P, f"{pfx}_fc2wT")
            for fc in range(KC):
                ps = pg.tile([128, dn], F32, name="f2ps", tag="pg")
                for kb in range(6):
                    wt = wpool.tile([128, 4, 128], BF16, name=f"f2w{kb}",
                                    tag="ws")
                    src = f2T[kb * 512:(kb + 1) * 512,
                              fc * 128:(fc + 1) * 128]
                    nc.sync.dma_start(
                        out=wt[:], in_=src.rearrange("(n p) m -> p n m", p=128))
                    for j in range(4):
                        k = kb * 4 + j
                        nc.tensor.matmul(ps[:], wt[:, j, :], vw(mlp[k][:]),
                                         start=(k == 0), stop=False)
                uwt = wpool.tile([ADIM, 128], BF16, name=f"uw{fc}", tag="ws")
                nc.sync.dma_start(out=uwt[:],
                                  in_=upT[:, fc * 128:(fc + 1) * 128])
                nc.tensor.matmul(ps[:], uwt[:], ah[:], start=False, stop=True)
                nc.vector.scalar_tensor_tensor(
                    vw(ch["st"][fc][:]), pv(ps[:]),
                    bias[f"{pfx}_fc2b"][:, fc:fc + 1],
                    vw(ch["st"][fc][:].bitcast(F32)), OP.add, OP.add)

    upto = "attn" if stage == "attn" else "full"
    xchunks = [{"st": xfm[b], "nc": NX, "bt": [(0, NX, NX)]}
               for b in range(NB)]
    def _yvw(a):
        return a.rearrange("p (b c) -> p b c", b=2)[:, :, 0:NYR]

    def _ypv(a):
        return a.rearrange("p (b c) -> p b c", b=2)

    ychunks = [{"st": yfm, "nc": 2 * NYP, "dn": 2 * NYR,
                "vw": _yvw, "pv": _ypv,
                "bt": [(0, NYP, NYR), (NYP, NYP, NYR)]}]
    stream("s", xchunks, upto)
    for b in range(NB):
        for k in range(KC):
            nc.sync.dma_start(out=P.xout[b, k * 128:(k + 1) * 128, :],
                              in_=xfm[b][k][:].bitcast(F32))
    stream("r", ychunks, upto)
    for b in range(NB):
        for k in range(KC):
            nc.sync.dma_start(
                out=P.yout[b, k * 128:(k + 1) * 128, :],
                in_=yfm[k][:, b * NYP:b * NYP + NYR].bitcast(F32))
    ctx.close()


# ============================ host side ============================

_CACHE = {}


def _get_nc(stage="full"):
    if stage not in _CACHE:
        _CACHE[stage] = build(stage)
    return _CACHE[stage]


def _prep_core_inputs(c, x, y, latents, scale_a, scale_v, W):
    b0 = 2 * c
    xb = x[b0:b0 + 2]
    yb = y[b0:b0 + 2]
    yfm = np.zeros((NB, D, NYP), np.float32)
    yfm[:, :, :NYR] = yb.transpose(0, 2, 1)
    m = {
        "xfm": np.ascontiguousarray(xb.transpose(0, 2, 1)),
        "yfm": yfm,
        "xrm": np.ascontiguousarray(xb),
        "yrm": np.ascontiguousarray(yb),
    }
    m.update(W)
    return m


def _shared_inputs(latents, scale_a, scale_v, kw):
    W = {
        "latfm": np.ascontiguousarray(latents[0].T),
        "ident": np.eye(128, dtype=np.float32),
        "ones": np.ones((128, 128), np.float32),
        "sa": np.array([[float(scale_a[0])]], np.float32),
        "sv": np.array([[float(scale_v[0])]], np.float32),
    }
    for p, q in (("s", "spec"), ("r", "rgb")):
        g = lambda nm: np.asarray(kw[f"{q}_{nm}"], np.float32)
        sc = float(g("scale")[0])
        import ml_dtypes
        bf = ml_dtypes.bfloat16
        W[f"{p}_qkvwT"] = np.ascontiguousarray(g("qkv_w").T.astype(bf))
        W[f"{p}_projwT"] = np.ascontiguousarray(g("proj_w").T.astype(bf))
        W[f"{p}_fc1wT"] = np.ascontiguousarray(g("fc1_w").T.astype(bf))
        W[f"{p}_fc2wT"] = np.ascontiguousarray(g("fc2_w").T.astype(bf))
        W[f"{p}_downwT"] = np.ascontiguousarray(g("down_w").T.astype(bf))
        W[f"{p}_upwT"] = np.ascontiguousarray((g("up_w").T * sc).astype(bf))
        W[f"{p}_qkvb"] = np.ascontiguousarray(g("qkv_b").reshape(18, 128).T)
        W[f"{p}_projb"] = np.ascontiguousarray(g("proj_b").reshape(6, 128).T)
        W[f"{p}_fc1b"] = np.ascontiguousarray(g("fc1_b").reshape(24, 128).T)
        W[f"{p}_fc2b"] = np.ascontiguousarray(
            (g("fc2_b") + g("up_b") * sc).reshape(6, 128).T)
        W[f"{p}_downb"] = np.ascontiguousarray(
            (g("down_b") * -1.702).reshape(ADIM, 1))
        W[f"{p}_downb2"] = np.ascontiguousarray(g("down_b").reshape(ADIM, 1))
        W[f"{p}_ln1g"] = np.ascontiguousarray(g("ln1_g").reshape(6, 128).T)
        W[f"{p}_ln1b"] = np.ascontiguousarray(g("ln1_b").reshape(6, 128).T)
        W[f"{p}_ln2g"] = np.ascontiguousarray(g("ln2_g").reshape(6, 128).T)
        W[f"{p}_ln2b"] = np.ascontiguousarray(g("ln2_b").reshape(6, 128).T)
    return W


def run(stage="full", trace=False, **kw):
    x = np.asarray(kw["x"], np.float32)
    y = np.asarray(kw["y"], np.float32)
    W = _shared_inputs(np.asarray(kw["latents"], np.float32),
                       np.asarray(kw["scale_a"], np.float32),
                       np.asarray(kw["scale_v"], np.float32), kw)
    in_maps = [_prep_core_inputs(c, x, y, None, None, None, W)
               for c in range(8)]
    nc = _get_nc(stage)
    res = run_bass_kernel_spmd(nc, in_maps, core_ids=list(range(8)),
                               trace=trace)
    xs, ys = [], []
    for c in range(8):
        r = res.results[c]
        xs.append(r["xout"].transpose(0, 2, 1))
        ys.append(r["yout"].transpose(0, 2, 1))
    xo = np.concatenate(xs, axis=0)
    yo = np.concatenate(ys, axis=0)
    return (xo, yo), res


def kernel(**inputs):
    out, _ = run(stage="full", trace=False, **inputs)
    return out
